# revision 22
# baseline (speedup 1.0000x reference)
"""Distributed causal multi-head attention block for 8 TRN2 NeuronCores.

Problem: y = proj(softmax_causal((x Wq)(x Wk)^T / 8) (x Wv)) with
B=1, S=4096, D=1024, H=16 heads, Dh=64, all float32.

Sharding (head-parallel attention + sequence-parallel projection):
- Each core c owns heads {2c, 2c+1}: it projects the FULL sequence through
  its 128 columns of Wq/Wk/Wv (x is replicated, transposed on host), runs
  causal attention for its two heads over all 4096 queries, and normalizes
  by the softmax denominator.
- A2A #1 (after chunk 6) re-shards chunks 0-6 head-major -> sequence-major;
  it hides under chunk-7 attention. Core c projects its 512 rows (chunk c).
- A2A #2 (after chunk 7, [8,128,64]) re-shards chunk 7's 512 queries
  8-ways; every core projects a 64-row share, overlapping A2A #2 with the
  own-chunk projection. Host stitches [512 own rows | 64 shared rows].

Scheduling: QKV-projection matmuls of chunk c+1 are emitted as filler
between the score and AV matmuls of chunk c's attention so the PE never
idles (keeps it at the 2.4 GHz p-state). Scores/probs/AV run in bf16 with
fp32 PSUM; the softmax denominator is produced by 64 ones-columns appended
to V (free M-width), normalized with reciprocal_approx_fast. Diagonal key
tiles only compute the live (q >= key-block) query range.
"""

import sys

sys.path.insert(0, "/opt/trn_rl_repo")

import numpy as np
import ml_dtypes

from concourse import bacc, tile, mybir
from concourse import bass_utils
from concourse.bass_utils import run_bass_kernel_spmd

bass_utils.upload_artifacts = lambda tmpdir: tmpdir  # no S3 in this container

dt = mybir.dt
AF = mybir.ActivationFunctionType

N_CORES = 8
S = 4096
D = 1024
P = 128
CH = 512            # seq chunk (query block per iteration)
NCHUNK = S // CH    # 8
KT_PER_CH = CH // P  # 4
OUT_ROWS = CH + 128  # own chunk + 1/8 shares of chunks 6 and 7

_BUILD_CACHE = {}


def _build(has_bq: bool, has_bp: bool):
    key = (has_bq, has_bp)
    if key in _BUILD_CACHE:
        return _BUILD_CACHE[key]

    nc = bacc.Bacc("TRN2", target_bir_lowering=False, debug=False,
                   num_devices=N_CORES)

    f32, f32r, bf16 = dt.float32, dt.float32r, dt.bfloat16

    # ---- external I/O (per-core values supplied via in_maps) ----
    xT_ext = nc.dram_tensor("xT", [NCHUNK, NCHUNK, P, CH], bf16, kind="ExternalInput")
    wq_ext = nc.dram_tensor("wq", [NCHUNK, P, P], bf16, kind="ExternalInput")
    wk_ext = nc.dram_tensor("wk", [NCHUNK, P, P], bf16, kind="ExternalInput")
    wv_ext = nc.dram_tensor("wv", [NCHUNK, P, P], bf16, kind="ExternalInput")
    wp_ext = nc.dram_tensor("wp", [NCHUNK, P, D], bf16, kind="ExternalInput")
    bq_ext = nc.dram_tensor("bq", [P, 3], f32, kind="ExternalInput")
    bp_ext = nc.dram_tensor("bp", [1, D], f32, kind="ExternalInput")
    bv_ext = nc.dram_tensor("bv", [1, P], bf16, kind="ExternalInput")
    out_ext = nc.dram_tensor("out", [OUT_ROWS, D], f32, kind="ExternalOutput")

    # triangular mask for the first 128 queries of each diagonal key tile,
    # duplicated for the two heads: mask[k, h, q] = 1 if k <= q else 0
    kk = np.arange(P)[:, None, None]
    qq = np.arange(P)[None, None, :]
    masks_np = np.broadcast_to(kk <= qq, (P, 2, P)).astype(ml_dtypes.bfloat16)
    masks_dram = nc.inline_tensor(np.ascontiguousarray(masks_np),
                                  name="masks_const")
    ones_np = np.ones((1, P), dtype=np.float32)
    ones_dram = nc.inline_tensor(ones_np, name="ones_const")

    with tile.TileContext(nc) as tc:
        with tc.tile_pool(name="const", bufs=1) as const, \
             tc.tile_pool(name="wpool", bufs=1) as wpool, \
             tc.tile_pool(name="resid", bufs=1) as resid, \
             tc.tile_pool(name="xp", bufs=32) as xp, \
             tc.tile_pool(name="probs", bufs=8) as probsp, \
             tc.tile_pool(name="small", bufs=3) as smallp, \
             tc.tile_pool(name="attnp", bufs=6) as attnp, \
             tc.tile_pool(name="outp", bufs=4) as outpool, \
             tc.tile_pool(name="psS", bufs=2, space="PSUM") as psS, \
             tc.tile_pool(name="psV", bufs=3, space="PSUM") as psV, \
             tc.tile_pool(name="psA", bufs=1, space="PSUM") as psA, \
             tc.tile_pool(name="dram", bufs=1, space="DRAM") as dram:

            # ---- resident weights / residency tiles ----
            wq_sb = wpool.tile([P, NCHUNK, P], bf16)
            wk_sb = wpool.tile([P, NCHUNK, P], bf16)
            wv_sb = wpool.tile([P, NCHUNK, P], bf16)
            wp_sb = wpool.tile([P, NCHUNK, D], bf16)
            masks_sb = const.tile([P, 2, P], bf16)

            qkt_tiles = []  # [128, 1024] bf16: cols 0:512 Q^T, 512:1024 K^T
            v_tiles = []    # [128, 4, 2, 128]: per ktile/head cols 0:64 V,
                            # 64:128 ones (denominator columns)
            for c in range(NCHUNK):
                qkt_tiles.append(resid.tile([P, 2 * CH], bf16, name=f"qkt{c}"))
                v_tiles.append(resid.tile([P, KT_PER_CH, 2, P], bf16,
                                          name=f"v{c}"))
            at_sb = resid.tile([P, NCHUNK, CH], bf16, name="at_sb")
            at6_sb = resid.tile([P, NCHUNK, 64], bf16, name="at6_sb")
            at7_sb = resid.tile([P, NCHUNK, 64], bf16, name="at7_sb")

            # A2A bounce buffers
            a2a1_in = dram.tile([N_CORES, P, CH], bf16)
            a2a1_out = dram.tile([N_CORES, P, CH], bf16)
            a2a2a_in = dram.tile([N_CORES, P, 64], bf16)
            a2a2a_out = dram.tile([N_CORES, P, 64], bf16)
            a2a2b_in = dram.tile([N_CORES, P, 64], bf16)
            a2a2b_out = dram.tile([N_CORES, P, 64], bf16)

            # ---- startup: x chunk 0 + weights first, then consts ----
            def emit_x_loads(cn):
                tiles = []
                for t in range(NCHUNK):
                    xt = xp.tile([P, CH], bf16, tag="x", name=f"x{cn}_{t}")
                    nc.sync.dma_start(xt[:], xT_ext.ap()[cn, t])
                    tiles.append(xt)
                return tiles

            x0_tiles = []
            for t in range(NCHUNK):
                xt = xp.tile([P, CH], bf16, tag="x", name=f"x0_{t}")
                nc.sync.dma_start(xt[:], xT_ext.ap()[0, t])
                nc.sync.dma_start(wq_sb[:, t, :], wq_ext.ap()[t])
                x0_tiles.append(xt)
            for t in range(NCHUNK):
                nc.sync.dma_start(wk_sb[:, t, :], wk_ext.ap()[t])
            nc.sync.dma_start(masks_sb[:], masks_dram.ap())
            for t in range(NCHUNK):
                nc.sync.dma_start(wv_sb[:, t, :], wv_ext.ap()[t])

            if has_bp:
                ones_r_sb = const.tile([1, P], f32r)
                nc.sync.dma_start(ones_r_sb[:],
                                  ones_dram.ap()[0:1, :].bitcast(f32r))
                bp_sb = const.tile([1, D], f32r)
                nc.sync.dma_start(bp_sb[:], bp_ext.ap().bitcast(f32r))
            if has_bq:
                bq_sb = const.tile([P, 3], f32)
                nc.sync.dma_start(bq_sb[:], bq_ext.ap())
                ones_bf_sb = const.tile([1, P], bf16)
                nc.vector.memset(ones_bf_sb[:], 1.0)
                bv_sb = const.tile([1, P], bf16)
                nc.sync.dma_start(bv_sb[:], bv_ext.ap())

            # denominator ones-column (col 64) is static; set once
            for c in range(NCHUNK):
                nc.vector.memset(v_tiles[c][:, :, :, 64:65], 1.0)

            def evict(dst_ap, src_ap, bias_ap=None):
                if bias_ap is not None:
                    nc.scalar.activation(dst_ap, src_ap, AF.Copy, bias=bias_ap)
                else:
                    nc.vector.tensor_copy(dst_ap, src_ap)

            # ---- phase-a (QKV projection) work units; used as PE filler ----
            def make_phase_a_units(cn, xap):
                units = []
                cell = {}

                def mk_qk(which, t):
                    def u():
                        if t == 0:
                            cell[which] = psA.tile([P, CH], f32, tag="qkv",
                                                   name=f"ps{which}{cn}")
                        w_sb = wq_sb if which == "q" else wk_sb
                        nc.tensor.matmul(cell[which][:], w_sb[:, t, :],
                                         xap(t)[:],
                                         start=(t == 0),
                                         stop=(t == NCHUNK - 1))
                        if t == NCHUNK - 1:
                            col = 0 if which == "q" else 1
                            lo = 0 if which == "q" else CH
                            evict(qkt_tiles[cn][:, lo:lo + CH], cell[which][:],
                                  bq_sb[:, col][:, None] if has_bq else None)
                    return u

                def mk_v(b, t):
                    def u():
                        if t == 0:
                            cell[b] = psA.tile([P, P], f32, tag="qkv",
                                               name=f"psv{cn}_{b}")
                            if has_bq:
                                nc.tensor.matmul(cell[b][:], ones_bf_sb[0:1, :],
                                                 bv_sb[0:1, :], start=True,
                                                 stop=False)
                        nc.tensor.matmul(
                            cell[b][:], xap(t)[:, P * b:P * (b + 1)],
                            wv_sb[:, t, :],
                            start=(t == 0 and not has_bq),
                            stop=(t == NCHUNK - 1))
                        if t == NCHUNK - 1:
                            nc.vector.tensor_copy(v_tiles[cn][:, b, 0, 0:64],
                                                  cell[b][:, 0:64])
                            nc.vector.tensor_copy(v_tiles[cn][:, b, 1, 0:64],
                                                  cell[b][:, 64:P])
                    return u

                early = []   # Q (needed at the chunk's kt 0) + half of V
                late = []    # K + rest of V (needed only at the diagonal)
                for t in range(NCHUNK):
                    early.append(mk_qk("q", t))
                for b in range(2):
                    for t in range(NCHUNK):
                        early.append(mk_v(b, t))
                for t in range(NCHUNK):
                    late.append(mk_qk("k", t))
                for b in range(2, KT_PER_CH):
                    for t in range(NCHUNK):
                        late.append(mk_v(b, t))
                return early, late

            # ---- attention for one query chunk, with filler interleave ----
            def emit_chunk(c, fillers, fstart=0):
                nkt = KT_PER_CH * (c + 1)
                av = [psV.tile([P, CH], f32, tag="av", name=f"av{c}_{h}")
                      for h in range(2)]
                pend_av = []
                nfill = len(fillers)
                fi = 0
                for kt in range(nkt):
                    kc, kb = divmod(kt, KT_PER_CH)
                    diag = (kc == c)
                    j0 = P * kb if diag else 0
                    sc = psS.tile([P, 2, CH], f32, tag="sc",
                                  name=f"sc{c}_{kt}")
                    for h in range(2):
                        lo, hi = 64 * h, 64 * h + 64
                        nc.tensor.matmul(
                            sc[:, h, j0:CH],
                            qkt_tiles[kc][lo:hi, CH + P * kb:CH + P * (kb + 1)],
                            qkt_tiles[c][lo:hi, j0:CH],
                            start=True, stop=True)
                    pr = probsp.tile([P, 2, CH], bf16, tag="pr")
                    nc.scalar.activation(pr[:, :, j0:CH], sc[:, :, j0:CH],
                                         AF.Exp, scale=0.125)
                    if diag:
                        nc.vector.tensor_mul(pr[:, :, j0:j0 + P],
                                             pr[:, :, j0:j0 + P],
                                             masks_sb[:])
                    # filler: next chunk's projection matmuls keep the PE hot
                    # while exp(kt) runs on the scalar engine
                    if kt < fstart:
                        want = 0
                    else:
                        want = min(nfill,
                                   ((kt - fstart + 2) * nfill)
                                   // (nkt - fstart))
                    while fi < want:
                        fillers[fi]()
                        fi += 1
                    if len(pend_av) >= 2:
                        pend_av.pop(0)()

                    def mk_av(kc=kc, kb=kb, j0=j0, pr=pr, kt=kt):
                        def u():
                            for h in range(2):
                                nc.tensor.matmul(
                                    av[h][0:65, j0:CH],
                                    v_tiles[kc][:, kb, h, 0:65],
                                    pr[:, h, j0:CH],
                                    start=(kt == 0), stop=(kt == nkt - 1),
                                    skip_group_check=True)
                        return u

                    pend_av.append(mk_av())
                for f in pend_av:
                    f()
                pend_av = []
                # normalize: row 64 of av holds the denominator. DVE ops
                # cannot shift partitions and the custom reciprocal only
                # works at base partition 0: stage av to SBUF (freeing the
                # PSUM slot for the next chunk ASAP -- emitted before the
                # leftover-filler drain), DMA-shift the denominator down,
                # recip, broadcast, then scale the AV rows.
                s_full = []
                for h in range(2):
                    sf = smallp.tile([P, CH], f32, tag=f"sf{h}")
                    nc.vector.tensor_copy(sf[0:65, :], av[h][0:65, :])
                    s_full.append(sf)
                dns = []
                for h in range(2):
                    dn = smallp.tile([1, CH], f32, tag=f"dn{h}")
                    nc.sync.dma_start(dn[:], s_full[h][64:65, :])
                    dns.append(dn)
                while fi < nfill:
                    fillers[fi]()
                    fi += 1
                for h in range(2):
                    r0 = smallp.tile([1, CH], f32, tag=f"r0{h}")
                    nc.vector.reciprocal_approx_fast(r0[:], dns[h][:])
                    rb = smallp.tile([64, CH], f32, tag=f"rb{h}")
                    nc.gpsimd.partition_broadcast(rb[:], r0[:])
                    attn = attnp.tile([64, CH], bf16, tag="attn")
                    nc.vector.tensor_mul(attn[:], s_full[h][0:64, :], rb[:])
                    if c <= NCHUNK - 3:
                        nc.sync.dma_start(a2a1_in[c, 64 * h:64 * h + 64, :],
                                          attn[:])
                        if c == NCHUNK - 3:
                            # duplicate chunk 5 into pieces 6,7 so every A2A
                            # piece is defined (those cores' copies unused)
                            for j in (NCHUNK - 2, NCHUNK - 1):
                                nc.sync.dma_start(
                                    a2a1_in[j, 64 * h:64 * h + 64, :],
                                    attn[:])
                    else:
                        dst = a2a2a_in if c == NCHUNK - 2 else a2a2b_in
                        nc.sync.dma_start(
                            dst[:, 64 * h:64 * h + 64, :]
                            .rearrange("j p q -> p j q"),
                            attn[:].rearrange("p (j q) -> p j q", j=N_CORES))

            # ---- main schedule ----
            early0, late0 = make_phase_a_units(0, lambda t: x0_tiles[t])
            for u in early0 + late0:
                u()
            x_next = emit_x_loads(1)
            early_next, late_next = make_phase_a_units(
                1, lambda t, x_next=x_next: x_next[t])
            pending_late = []
            for c in range(NCHUNK):
                # chunk c interleaves: chunk c+1's Q/V-half (must finish
                # here) plus chunk c's deferred K/V-half (needed only by
                # this chunk's diagonal tiles, paced to land in time)
                emit_chunk(c, pending_late + early_next)
                pending_late = late_next
                if c + 2 < NCHUNK:
                    x_next = emit_x_loads(c + 2)
                    early_next, late_next = make_phase_a_units(
                        c + 2, lambda t, x_next=x_next: x_next[t])
                else:
                    early_next, late_next = [], []
                if c == 1:
                    for t in range(NCHUNK):
                        nc.sync.dma_start(wp_sb[:, t, :], wp_ext.ap()[t])
                if c == NCHUNK - 3:
                    # A2A #1 (chunks 0-5) hides under chunks 6+7 attention
                    nc.gpsimd.collective_compute(
                        "AllToAll", mybir.AluOpType.bypass,
                        ins=[a2a1_in[:]], outs=[a2a1_out[:]],
                        replica_groups=[list(range(N_CORES))],
                    )
                if c == NCHUNK - 2:
                    # A2A #2a: chunk 6 queries re-sharded 8 ways (tiny)
                    nc.gpsimd.collective_compute(
                        "AllToAll", mybir.AluOpType.bypass,
                        ins=[a2a2a_in[:]], outs=[a2a2a_out[:]],
                        replica_groups=[list(range(N_CORES))],
                    )
                    # emit these loads AFTER chunk 6 so they queue behind its
                    # normalize DMAs -- ahead of them they head-of-line block
                    # the av-slot recycling chain and stall chunk-7 attention
                    for t in range(NCHUNK):
                        nc.sync.dma_start(at_sb[:, t, :], a2a1_out[t])
                    nc.sync.dma_start(at6_sb[:],
                                      a2a2a_out[:].rearrange("j p q -> p j q"))

            # ---- A2A #2b: chunk 7 queries re-sharded 8 ways ----
            nc.gpsimd.collective_compute(
                "AllToAll", mybir.AluOpType.bypass,
                ins=[a2a2b_in[:]], outs=[a2a2b_out[:]],
                replica_groups=[list(range(N_CORES))],
            )
            nc.sync.dma_start(at7_sb[:],
                              a2a2b_out[:].rearrange("j p q -> p j q"))

            # ---- output projection (own rows overlap A2A #2b) ----
            def emit_proj(po, rows, lhs_for_t, out_row0):
                for dc in range(2):
                    dsl = slice(CH * dc, CH * (dc + 1))
                    if has_bp:
                        nc.tensor.matmul(po[0:rows, dc, :],
                                         ones_r_sb[0:1, 0:rows],
                                         bp_sb[0:1, dsl], start=True,
                                         stop=False)
                    for t in range(NCHUNK):
                        nc.tensor.matmul(po[0:rows, dc, :], lhs_for_t(t),
                                         wp_sb[:, t, dsl],
                                         start=(t == 0 and not has_bp),
                                         stop=(t == NCHUNK - 1))
                    o_sb = outpool.tile([P, CH], f32, tag="out")
                    nc.vector.tensor_copy(o_sb[0:rows, :], po[0:rows, dc, :])
                    nc.sync.dma_start(
                        out_ext.ap()[out_row0:out_row0 + rows, dsl],
                        o_sb[0:rows, :])

            # own-chunk proj qs 0-2, then the chunk-6 share, then qs 3 as
            # always-ready PE work covering A2A #2b's latency, then share 7
            for qs in range(3):
                qsl = slice(P * qs, P * (qs + 1))
                po = psS.tile([P, 2, CH], f32, tag="sc", name=f"po{qs}")
                emit_proj(po, P, lambda t, qsl=qsl: at_sb[:, t, qsl], P * qs)
            po6 = psS.tile([P, 2, CH], f32, tag="sc", name="po6")
            emit_proj(po6, 64, lambda t: at6_sb[:, t, :], CH)
            qsl3 = slice(P * 3, P * 4)
            po3 = psS.tile([P, 2, CH], f32, tag="sc", name="po3")
            emit_proj(po3, P, lambda t: at_sb[:, t, qsl3], P * 3)
            po7 = psS.tile([P, 2, CH], f32, tag="sc", name="po7")
            emit_proj(po7, 64, lambda t: at7_sb[:, t, :], CH + 64)

    nc.compile()
    _BUILD_CACHE[key] = nc
    return nc


def _prep_in_maps(x, Wqkv, bqkv, Wproj, bproj):
    x = np.asarray(x, dtype=np.float32)
    Wqkv = np.asarray(Wqkv, dtype=np.float32)
    bqkv = np.asarray(bqkv, dtype=np.float32)
    Wproj = np.asarray(Wproj, dtype=np.float32)
    bproj = np.asarray(bproj, dtype=np.float32)
    xT = np.ascontiguousarray(
        x.reshape(S, D).T.astype(ml_dtypes.bfloat16)
        .reshape(NCHUNK, P, NCHUNK, CH).transpose(2, 0, 1, 3))
    bp = np.ascontiguousarray(bproj.reshape(1, D))
    in_maps = []
    for i in range(N_CORES):
        sl = slice(P * i, P * (i + 1))
        bq = np.stack([bqkv[P * i:P * (i + 1)],
                       bqkv[D + P * i:D + P * (i + 1)],
                       bqkv[2 * D + P * i:2 * D + P * (i + 1)]], axis=1)
        in_maps.append({
            "xT": xT,
            "wq": np.ascontiguousarray(Wqkv[:, sl].astype(ml_dtypes.bfloat16).reshape(NCHUNK, P, P)),
            "wk": np.ascontiguousarray(Wqkv[:, D + P * i:D + P * (i + 1)].astype(ml_dtypes.bfloat16).reshape(NCHUNK, P, P)),
            "wv": np.ascontiguousarray(Wqkv[:, 2 * D + P * i:2 * D + P * (i + 1)].astype(ml_dtypes.bfloat16).reshape(NCHUNK, P, P)),
            "wp": np.ascontiguousarray(Wproj.astype(ml_dtypes.bfloat16).reshape(NCHUNK, P, D)),
            "bq": np.ascontiguousarray(bq),
            "bv": bqkv[2 * D + P * i:2 * D + P * (i + 1)].reshape(1, P).astype(ml_dtypes.bfloat16),
            "bp": bp,
        })
    return in_maps


def _run(x, Wqkv, bqkv, Wproj, bproj, trace=False):
    nc = _build(bool(np.any(np.asarray(bqkv))), bool(np.any(np.asarray(bproj))))
    in_maps = _prep_in_maps(x, Wqkv, bqkv, Wproj, bproj)
    res = run_bass_kernel_spmd(nc, in_maps, core_ids=list(range(N_CORES)),
                               trace=trace)
    full = np.empty((S, D), dtype=np.float32)
    for c in range(N_CORES):
        if c < NCHUNK - 2:
            full[CH * c:CH * (c + 1)] = res.results[c]["out"][0:CH]
        full[CH * (NCHUNK - 2) + 64 * c:CH * (NCHUNK - 2) + 64 * (c + 1)] = \
            res.results[c]["out"][CH:CH + 64]
        full[CH * (NCHUNK - 1) + 64 * c:CH * (NCHUNK - 1) + 64 * (c + 1)] = \
            res.results[c]["out"][CH + 64:CH + 128]
    return full.reshape(1, S, D), res


def kernel(x, Wqkv, bqkv, Wproj, bproj):
    out, _ = _run(x, Wqkv, bqkv, Wproj, bproj, trace=False)
    return out
